# revision 1
# baseline (speedup 1.0000x reference)
"""Deformable Conv (DCNv1) Trainium2 Bass kernel — v2.

Problem: x[4,64,128,128], offset[4,18,128,128], weight[64,64,3,3], bias[64]
-> out[4,64,128,128].  3x3 deformable conv, stride 1, pad 1, bilinear sampling.

Sharding: 8 cores = (batch b = core//2) x (H-half h = core%2). Each core
computes out[b, :, 64h:64h+64, :].

Per-core algorithm (v2 — gather-centric redesign):
  1. DRAM table tbl[(y', x'), (yc, xc, c)] bf16, 512B rows: row (y', x')
     holds the full 2x2 bilinear corner block (y-pair x x-pair) for all 64
     channels.  Built with 6 large DMAs from a host-pretransposed x-major
     image tile (no per-row DMAs).
  2. Index math on DVE in the gather's wrapped int16 layout: one index per
     (k, wo) sample = table row y'*160 + x'.
  3. Main loop over 64 output rows: ONE transpose-mode dma_gather per row
     (1152 idx x 512B) round-robin over 4 SWDGE queues (queue q is served
     by Q7 core pair q, so descriptor generation overlaps 4-wide).
     Output lands as v_T[(xc,c), yc, (k,wo)] — GEMM-ready, no transposes.
  4. Corner weights: PE broadcast-matmul sel2[2,128] x RH[2ho:2ho+2, 2304]
     -> WG[(xc,c), (yc,k,wo)] in PSUM (c-replication done by the PE, not
     DVE).  RH (wy*wx products for every ho) is precomputed once on DVE
     from host-supplied ho-major offset copies.
  5. DVE: m = v_T * WG (one 2304-wide multiply), fold yc (one add).
  6. GEMM: 9 chunks contract (xc, c)=128 with host-replicated weights
     wt2[(xc,c), (k,o)]; the xc corner fold happens inside the PE in fp32.
"""

import numpy as np
import sys

sys.path.insert(0, "/opt/trn_rl_repo")

import concourse.bacc as bacc
import concourse.bass as bass
import concourse.mybir as mybir
from concourse import tile
from concourse.bass_utils import run_bass_kernel_spmd
from concourse.masks import make_identity

from concourse.library_config import mlp

# problem constants
B, C, H, W = 4, 64, 128, 128
K, O = 9, 64
HO2 = 64            # output rows per core
NYE = 93            # xs rows (img rows 64h-18 .. 64h+74, zero padded)
NYT = 92            # table y' rows
WP = 160            # table x' cols (img cols -16 .. 143)
TROWS = NYT * WP
RL = 4 * C          # 256 elems (512B) per table row
F32 = mybir.dt.float32
BF16 = mybir.dt.bfloat16
I32 = mybir.dt.int32
I16 = mybir.dt.int16
AX = mybir.AluOpType

_CACHE = {}


def _build_nc():
    nc = bacc.Bacc("TRN2", target_bir_lowering=False, debug=False,
                   num_swdge_queues=4)

    xs_t = nc.dram_tensor("xs_t", [W, NYE * C], F32, kind="ExternalInput").ap()
    offs_w2 = nc.dram_tensor("offs_w2", [128, 1152], F32, kind="ExternalInput").ap()
    wt2 = nc.dram_tensor("wt2", [128, 5 * O], F32, kind="ExternalInput").ap()
    bias_d = nc.dram_tensor("bias", [O, 1], F32, kind="ExternalInput").ap()
    out_d = nc.dram_tensor("out", [O, HO2, W], F32, kind="ExternalOutput").ap()
    tbl = nc.dram_tensor("tbl", [TROWS, RL], BF16, kind="Internal").ap()

    with tile.TileContext(nc) as tc:
        with (
            tc.tile_pool(name="consts", bufs=1) as consts,
            tc.tile_pool(name="wma", bufs=6) as wma,
            tc.tile_pool(name="wmb", bufs=10) as wmb,
            tc.tile_pool(name="wmc", bufs=1) as wmc,
            tc.tile_pool(name="v4p", bufs=8) as v4p,
            tc.tile_pool(name="mp", bufs=3) as mp,
            tc.tile_pool(name="sp", bufs=4) as sp,
            tc.tile_pool(name="outp", bufs=4) as outp,
            tc.tile_pool(name="ps_tr", bufs=3, space="PSUM") as ps_tr,
            tc.tile_pool(name="ps_mm", bufs=2, space="PSUM") as ps_mm,
        ):
            nc.gpsimd.load_library(mlp)

            # ---------------- load inputs ----------------
            # xs via HWDGE f32 halves + DVE cast (faster than SWDGE
            # cast-load, and keeps SWDGE lane 0 free for the gathers)
            ow2 = consts.tile([128, 1152], F32)
            nc.scalar.dma_start(ow2, offs_w2)
            xsf = consts.tile([W, NYE * C], F32)
            XH = (NYE * C) // 2
            nc.sync.dma_start(xsf[:, :XH], xs_t[:, :XH])
            nc.scalar.dma_start(xsf[:, XH:], xs_t[:, XH:])
            xsb = consts.tile([W, NYE * C], BF16)
            nc.vector.tensor_copy(xsb[:, :XH], xsf[:, :XH])
            nc.vector.tensor_copy(xsb[:, XH:], xsf[:, XH:])
            wtf = consts.tile([128, 5 * O], F32)
            nc.scalar.dma_start(wtf, wt2)
            bias_sb = consts.tile([O, 1], F32)
            nc.scalar.dma_start(bias_sb, bias_d)

            wt_bf = consts.tile([128, 5 * O], BF16)
            nc.vector.tensor_copy(wt_bf, wtf)
            ident = consts.tile([128, 128], BF16)
            make_identity(nc, ident)

            # ---------------- table build (disjoint zones, 2 queues) ----
            zr = consts.tile([128, C], BF16)
            nc.vector.memset(zr, 0.0)
            tbl3 = tbl.rearrange("(y x) e -> y x e", x=WP)
            tbl4 = tbl.rearrange("(y x) (s c) -> y x s c", x=WP, c=C)
            # zones never overlap, so sync/scalar can run concurrently:
            # sync:   x'<15 zeros, x'=15 xc0 zeros, bodies xc=0
            # scalar: x'>143 zeros, x'=143 xc1 zeros, bodies xc=1
            nc.sync.dma_start(
                tbl3[:, 0:15, :].rearrange("y x e -> y (x e)"),
                zr[:NYT, None, :].to_broadcast((NYT, 60, C)))
            nc.sync.dma_start(tbl4[:, 15, 0::2, :],
                              zr[:NYT, None, :].to_broadcast((NYT, 2, C)))
            nc.scalar.dma_start(
                tbl3[:, 144:160, :].rearrange("y x e -> y (x e)"),
                zr[:NYT, None, :].to_broadcast((NYT, 64, C)))
            nc.scalar.dma_start(tbl4[:, 143, 1::2, :],
                                zr[:NYT, None, :].to_broadcast((NYT, 2, C)))
            xsv = xsb.rearrange("x (y c) -> x y c", c=C)

            def table_bodies(ylo, yhi):
                for yc in range(2):
                    for xc in range(2):
                        xlo = 16 - xc   # x' = img_x + 16 - xc
                        dst = tbl3[ylo:yhi, xlo:xlo + 128,
                                   (2 * yc + xc) * C:(2 * yc + xc + 1) * C]
                        eng = nc.sync if xc == 0 else nc.scalar
                        eng.dma_start(dst.rearrange("y x c -> x y c"),
                                      xsv[:, ylo + yc:yhi + yc, :])

            table_bodies(0, 45)
            table_bodies(45, NYT)

            # ---------------- per-partition consts ----------------
            pio = wma.tile([128, 1], I32, tag="mini", name="pio")
            nc.gpsimd.iota(pio, pattern=[[0, 1]], base=0, channel_multiplier=1)
            # 8*(p//16) as float (wrapped idx math)
            g8i = wma.tile([128, 1], I32, tag="mini", name="g8i")
            nc.vector.tensor_scalar(g8i, pio, 4, 3, AX.arith_shift_right,
                                    AX.logical_shift_left)
            g8f = consts.tile([128, 1], F32)
            nc.vector.tensor_copy(g8f, g8i)
            # p%16 as float
            qi = wma.tile([128, 1], I32, tag="mini", name="qi")
            nc.vector.tensor_scalar(qi, pio, 15, None, AX.bitwise_and, AX.bypass)
            qf = consts.tile([128, 1], F32)
            nc.vector.tensor_copy(qf, qi)

            # ---------------- wrapped index math ----------------
            # layouts follow kernel v1: p = 16g+q handles ho-block g, lane q
            # ow2[p, (d, h, k, 2)]; iota consts in (d, h, k) layout
            ayi = wma.tile([128, 576], I32, tag="w576", name="ayi")
            nc.gpsimd.iota(ayi, pattern=[[0, 8], [1, 8], [1, 3], [0, 3]],
                           base=17, channel_multiplier=0)
            ayf = wma.tile([128, 576], F32, tag="w576", name="ayf")
            nc.vector.tensor_copy(ayf, ayi)
            axi = wma.tile([128, 576], I32, tag="w576", name="axi")
            nc.gpsimd.iota(axi, pattern=[[16, 8], [0, 8], [0, 3], [1, 3]],
                           base=15, channel_multiplier=0)
            axf = wma.tile([128, 576], F32, tag="w576", name="axf")
            nc.vector.tensor_copy(axf, axi)

            ow2v = ow2.rearrange("p (d h k two) -> p d h k two", d=8, h=8, two=2)
            pyw = wma.tile([128, 576], F32, tag="w576", name="pyw")
            nc.vector.scalar_tensor_tensor(pyw, ow2v[:, :, :, :, 0], g8f, ayf,
                                           op0=AX.add, op1=AX.add)
            pxw = wma.tile([128, 576], F32, tag="w576", name="pxw")
            nc.vector.scalar_tensor_tensor(pxw, ow2v[:, :, :, :, 1], qf, axf,
                                           op0=AX.add, op1=AX.add)
            # exact floor (HW rounds-to-nearest on f32->i32)
            ywi = wma.tile([128, 576], I32, tag="w576", name="ywi")
            nc.vector.tensor_copy(ywi, pyw)
            ywf = wma.tile([128, 576], F32, tag="w576", name="ywf")
            nc.vector.tensor_copy(ywf, ywi)
            crw = wma.tile([128, 576], F32, tag="w576", name="crw")
            nc.vector.tensor_tensor(crw, ywf, pyw, AX.is_gt)
            nc.vector.tensor_tensor(ywf, ywf, crw, AX.subtract)
            xwi = wma.tile([128, 576], I32, tag="w576", name="xwi")
            nc.vector.tensor_copy(xwi, pxw)
            xwf = wma.tile([128, 576], F32, tag="w576", name="xwf")
            nc.vector.tensor_copy(xwf, xwi)
            nc.vector.tensor_tensor(crw, xwf, pxw, AX.is_gt)
            nc.vector.tensor_tensor(xwf, xwf, crw, AX.subtract)
            # fracs BEFORE clamping (same floor tensors as the idx path)
            fyw = wma.tile([128, 576], F32, tag="w576", name="fyw")
            nc.vector.tensor_tensor(fyw, pyw, ywf, AX.subtract)
            fxw = wma.tile([128, 576], F32, tag="w576", name="fxw")
            nc.vector.tensor_tensor(fxw, pxw, xwf, AX.subtract)
            hyw = wma.tile([128, 576], F32, tag="w576", name="hyw")
            nc.vector.tensor_scalar(hyw, fyw, -1.0, 1.0, AX.mult, AX.add)
            hxw = wma.tile([128, 576], F32, tag="w576", name="hxw")
            nc.vector.tensor_scalar(hxw, fxw, -1.0, 1.0, AX.mult, AX.add)
            # clamp y' to [0, 90], x' to [0, 157]
            nc.vector.tensor_scalar(ywf, ywf, 0.0, 90.0, AX.max, AX.min)
            nc.vector.tensor_scalar(xwf, xwf, 0.0, 157.0, AX.max, AX.min)
            roww = wma.tile([128, 576], F32, tag="w576", name="roww")
            nc.vector.scalar_tensor_tensor(roww, ywf, float(WP), xwf,
                                           op0=AX.mult, op1=AX.add)
            # idxw [p, (h, k, d)] int16 (gather col = k*8 + d within block h)
            idxw = wmc.tile([128, 576], I16, tag="idxw", name="idxw")
            idxwv = idxw.rearrange("p (h k d) -> p h k d", h=8, k=9)
            nc.vector.tensor_copy(
                idxwv.rearrange("p h k d -> p d h k"), roww)

            # ---------------- corner-weight products (wrapped layout) ----
            # w4w[16g+q, (d, h, k, q4)], q4 = (yc, xc): hy*hx, hy*lx, ly*hx,
            # ly*lx.  Fracs come from the same pyw/pxw floors as the idx
            # path, so floor/weight can never disagree.
            # each product stored TWICE: the pair gives the loop's big
            # broadcast multiply a stride-1 innermost dim (DVE 2x mode)
            w4w = consts.tile([128, 4608], BF16)
            w4wv = w4w.rearrange("p (a q two) -> p a q two", q=4, two=2)
            for two in range(2):
                nc.vector.tensor_tensor(w4wv[:, :, 0, two], hyw, hxw, AX.mult)
                nc.vector.tensor_tensor(w4wv[:, :, 1, two], hyw, fxw, AX.mult)
                nc.vector.tensor_tensor(w4wv[:, :, 2, two], fyw, hxw, AX.mult)
                nc.vector.tensor_tensor(w4wv[:, :, 3, two], fyw, fxw, AX.mult)
            # table writes must complete before gathers (raw DRAM deps)
            tc.strict_bb_all_engine_barrier()

            # wrap-replicate idx + (g<->d) block-swap of w4 AFTER the
            # barrier: tile deps let early gathers start as soon as their
            # slices land, overlapping these DMAs with the loop head.
            wrapped = consts.tile([128, 4608], I16)
            w4o = consts.tile([128, 4608], BF16)

            def wrap_group(g):
                for rep in range(8):
                    eng = nc.sync if (g + rep) % 2 == 0 else nc.scalar
                    eng.dma_start(
                        wrapped[16 * rep:16 * rep + 16, g * 576:(g + 1) * 576],
                        idxw[16 * g:16 * g + 16, :])
                for d in range(8):
                    eng = nc.sync if (g + d) % 2 == 1 else nc.scalar
                    eng.dma_start(
                        w4o[16 * d:16 * d + 16, g * 576:(g + 1) * 576],
                        w4w[16 * g:16 * g + 16, d * 576:(d + 1) * 576])

            for g in range(8):
                wrap_group(g)

            # ---------------- main loop ----------------
            for ho in range(HO2):
                vt = v4p.tile([128, 9, 256], BF16)
                nc.gpsimd.dma_gather(
                    vt, tbl, wrapped[:, ho * 72:(ho + 1) * 72],
                    1152, 1152, RL, transpose=False, single_packet=False,
                    queue_num=ho % 4)
                # m = v4 * w4; weight pairs make the innermost dim a real
                # stride-1 run of 2 (DVE fast mode), c/2 broadcast above it
                v4v = vt.rearrange("p a (q c2 two) -> p a q c2 two",
                                   q=4, c2=C // 2, two=2)
                w4b = (w4o[:, ho * 72:(ho + 1) * 72]
                       .rearrange("p (k q two) -> p k q two", q=4, two=2)
                       [:, :, :, None, :].to_broadcast((128, K, 4, C // 2, 2)))
                m = mp.tile([128, 2304], BF16)
                m4 = m.rearrange("p (k q c) -> p k q c", q=4, c=C)
                m4b = m.rearrange("p (k q c2 two) -> p k q c2 two",
                                  k=K, q=4, c2=C // 2, two=2)
                with nc.allow_low_precision(reason="bilinear corner sum"):
                    nc.vector.tensor_tensor(m4b, v4v, w4b, AX.mult)
                    # fold 4 corners: two strided adds
                    s1 = sp.tile([128, 1152], BF16, tag="s1")
                    s1v = s1.rearrange("p (k t c) -> p k t c", t=2, c=C)
                    nc.vector.tensor_tensor(s1v, m4[:, :, 0:2, :],
                                            m4[:, :, 2:4, :], AX.add)
                    s = sp.tile([128, 576], BF16, tag="s")
                    nc.vector.tensor_tensor(
                        s.rearrange("p (k c) -> p k c", c=C),
                        s1v[:, :, 0, :], s1v[:, :, 1, :], AX.add)
                # transpose s -> s_T chunks; GEMM over (k, c) = 576
                st = outp.tile([128, 5 * 128], BF16, tag="st")
                for i in range(5):
                    cw = min(128, 576 - i * 128)
                    stps = ps_tr.tile([128, 128], BF16, tag="tr")
                    nc.tensor.transpose(stps[:cw, :],
                                        s[:, i * 128:i * 128 + cw], ident)
                    nc.scalar.copy(st[:cw, i * 128:(i + 1) * 128], stps[:cw, :])
                omm = ps_mm.tile([O, W], F32)
                for i in range(5):
                    cw = min(128, 576 - i * 128)
                    nc.tensor.matmul(
                        omm, wt_bf[:cw, i * O:(i + 1) * O],
                        st[:cw, i * 128:(i + 1) * 128],
                        start=(i == 0), stop=(i == 4))
                osb = outp.tile([O, W], F32, tag="osb")
                nc.vector.tensor_tensor(osb, omm,
                                        bias_sb.to_broadcast((O, W)), AX.add)
                nc.sync.dma_start(out_d[:, ho, :], osb)

    nc.compile()
    return nc


def _shard_inputs(x, offset, weight, bias):
    # wt2: 5 chunks of the [576=(k,c), O] weight, rows 128-chunked
    wtc = weight.reshape(O, C, K).transpose(2, 1, 0).reshape(576, O)
    wt2 = np.zeros((128, 5 * O), np.float32)
    for i in range(5):
        cw = min(128, 576 - i * 128)
        wt2[:cw, i * O:(i + 1) * O] = wtc[i * 128:i * 128 + cw]
    wt2 = np.ascontiguousarray(wt2, np.float32)
    b2 = np.ascontiguousarray(bias.reshape(O, 1), np.float32)
    in_maps = []
    for core in range(8):
        b, h = core // 2, core % 2
        # xs_t [x, (y, c)] zero-padded rows 64h-18 .. 64h+74
        xpad = np.zeros((NYE, W, C), np.float32)
        ylo = 64 * h - 18
        src_lo, src_hi = max(0, ylo), min(H, ylo + NYE)
        xpad[src_lo - ylo:src_hi - ylo] = x[b, :, src_lo:src_hi, :].transpose(1, 2, 0)
        xs_t = np.ascontiguousarray(
            xpad.transpose(1, 0, 2).reshape(W, NYE * C), np.float32)
        offs = offset[b, :, 64 * h:64 * h + 64, :]  # [18, 64, 128]
        # wrapped layout: [16g+q, (d, hl, ch)] = offs[ch, 8g+hl, 16d+q]
        ow = offs.transpose(1, 2, 0).reshape(8, 8, 8, 16, 18)  # [g, hl, d, q, ch]
        offs_w2 = np.ascontiguousarray(
            ow.transpose(0, 3, 2, 1, 4).reshape(128, 1152), np.float32)
        in_maps.append({"xs_t": xs_t, "offs_w2": offs_w2, "wt2": wt2,
                        "bias": b2})
    return in_maps


def kernel(x, offset, weight, bias):
    x = np.asarray(x, np.float32)
    offset = np.asarray(offset, np.float32)
    weight = np.asarray(weight, np.float32)
    bias = np.asarray(bias, np.float32)
    if "nc" not in _CACHE:
        _CACHE["nc"] = _build_nc()
    nc = _CACHE["nc"]
    in_maps = _shard_inputs(x, offset, weight, bias)
    res = run_bass_kernel_spmd(nc, in_maps, core_ids=list(range(8)),
                               trace=bool(_CACHE.get("trace")))
    _CACHE["exec_time_ns"] = res.exec_time_ns
    _CACHE["results"] = res
    full = np.zeros((B, O, H, W), np.float32)
    for core in range(8):
        b, h = core // 2, core % 2
        full[b, :, 64 * h:64 * h + 64, :] = res.results[core]["out"]
    return full


if __name__ == "__main__":
    import reference as ref
    inputs = {k: np.asarray(v) for k, v in ref.setup_inputs().items()}
    out = kernel(**inputs)
    exp = np.asarray(ref.reference(**inputs))
    print("rel:", np.abs(out - exp).max() / np.abs(exp).max())

